# revision 1
# baseline (speedup 1.0000x reference)
"""ConcatAttention Trainium2 kernel (8-core data-parallel over batch).

Computes, per batch row b:
    scores[b, l] = sum_h v[h] * tanh(q_proj[b, h] + (key_val[l, b] @ Wk)[h])
    out[b, 0, :] = softmax(scores[b, :])

Device-side per core (B_shard = 4 batch rows):
  - main matmul  kpT[h, l] = Wk^T @ keyT   (float32r, K=512 via 4 PSUM-accum chunks)
  - ACT fuses    energy = tanh(kpT + q_projT[h])  (per-partition bias)
  - v-dot        scores[1, l] = v^T @ energy      (M=1 matmuls, PSUM accum over h)
  - softmax on ACT/DVE, DMA out.

Host side only reshapes/shards: key_val is laid out [b][h_in][L] per core so the
device streams fully contiguous tiles (no on-chip transposes), and the tiny
q_proj = query @ Wq is precomputed on host (it is per-core constant bias data).
"""

import os
import sys

for _p in ("/opt/trn_rl_repo", os.path.expanduser("~/trn_rl_repo")):
    if os.path.isdir(_p) and _p not in sys.path:
        sys.path.insert(0, _p)

import numpy as np

L, B, H = 4096, 32, 512
NCORES = 8
BS = B // NCORES          # batch rows per core
P = 128
CI = H // P               # input-feature chunks (contraction)
CH = H // P               # output-feature chunks
LC = 512                  # l-tile (matmul moving free dim)
NLC = L // LC
QRT = L // 4              # key DMA granularity: [128, QRT] = 512 KiB
WARMUP_MM = 4             # dummy matmul groups to heat the PE HAM clock gate

_CACHE = {}


def _build_nc():
    import concourse.bacc as bacc
    import concourse.mybir as mybir
    import concourse.tile as tile

    f32 = mybir.dt.float32
    f32r = mybir.dt.float32r
    Act = mybir.ActivationFunctionType

    nc = bacc.Bacc("TRN2", target_bir_lowering=False)

    keyT = nc.dram_tensor("keyT", [BS, CI, P, L], f32r, kind="ExternalInput")
    wk = nc.dram_tensor("wk", [P, CI, H], f32r, kind="ExternalInput")
    qpT = nc.dram_tensor("qpT", [P, CH, BS], f32, kind="ExternalInput")
    vT = nc.dram_tensor("vT", [P, CH], f32r, kind="ExternalInput")
    # -U_b: softmax shift per batch row (host-derived safe bound near the
    # row max; softmax is invariant to the exact value)
    negu = nc.dram_tensor("negu", [1, BS], f32, kind="ExternalInput")
    out = nc.dram_tensor("out", [BS, L], f32, kind="ExternalOutput")

    with tile.TileContext(nc) as tc:
        with tc.tile_pool(name="singles", bufs=1) as singles, \
             tc.tile_pool(name="ktp", bufs=8) as ktp, \
             tc.tile_pool(name="enp", bufs=8) as enp, \
             tc.tile_pool(name="scrp", bufs=2) as scrp, \
             tc.tile_pool(name="kpp", bufs=6, space="PSUM") as kpp, \
             tc.tile_pool(name="scp", bufs=2, space="PSUM") as scp:

            def load_kt(b, plan, tiles=None, pos=0):
                """plan: list of l-slice widths; each slice is one joint DMA
                carrying all CI feature chunks."""
                if tiles is None:
                    tiles = []
                for w in plan:
                    t = ktp.tile([P, CI, QRT], f32r, tag="kt")
                    nc.sync.dma_start(
                        t[:, :, :w],
                        keyT[b, :, :, pos:pos + w].rearrange("c p l -> p c l"))
                    tiles.append((pos, w, t))
                    pos += w
                return tiles

            def kt_slice(tiles, ci, l0):
                for pos, w, t in tiles:
                    if pos <= l0 and l0 + LC <= pos + w:
                        return t[:, ci, l0 - pos:l0 - pos + LC]
                raise AssertionError("no tile covers slice")

            # ---- constants on the gpsimd queue (per-ci so the first matmul
            # group can start as soon as its chunk lands) ----
            wk_sb = singles.tile([P, CI, H], f32r, tag="wk")
            for ci in range(CI):
                nc.gpsimd.dma_start(wk_sb[:, ci, :], wk[:, ci, :])
            qpT_sb = singles.tile([P, CH, BS], f32, tag="qpT")
            nc.gpsimd.dma_start(qpT_sb, qpT[:, :, :])
            vT_sb = singles.tile([P, CH], f32r, tag="vT")
            nc.gpsimd.dma_start(vT_sb, vT[:, :])
            negu_sb = singles.tile([1, BS], f32, tag="negu")
            nc.gpsimd.dma_start(negu_sb, negu[:, :])
            kts = load_kt(0, [LC, LC, LC, LC, QRT, QRT])

            # ---- PE warmup: cheap dummy matmuls on zeros while the first
            # key tiles stream in, so the HAM clock gate reaches 2.4 GHz
            # before real work starts ----
            wu = singles.tile([P, LC], f32, tag="warmup")
            nc.vector.memset(wu, 0.0)
            wur = wu[:, :].bitcast(f32r)
            trash = singles.tile([1, 1], f32, tag="trash")
            for g in range(WARMUP_MM):
                wps = kpp.tile([P, LC], f32, tag="kp")
                for i in range(4):
                    nc.tensor.matmul(wps[:, 0:P], wur[:, 0:P], wur[:, 0:P],
                                     start=(i == 0), stop=(i == 3))
                nc.vector.tensor_copy(trash, wps[0:1, 0:1])

            def emit_vdot(b, lc, ens):
                sc = scp.tile([1, LC], mybir.dt.float32, tag="sc")
                for ch in range(CH):
                    nc.tensor.matmul(sc, vT_sb[:, ch:ch + 1], ens[ch],
                                     start=(ch == 0), stop=(ch == CH - 1))
                return sc

            for b in range(BS):
                # Chunked softmax with a fixed host-supplied shift U_b:
                # exp each chunk straight out of PSUM as it completes.
                scores = scrp.tile([1, L], f32, tag="scores")
                csums = singles.tile([1, NLC], f32, tag=f"csums_{b}")

                def finish_chunk(plc, pens, scores=scores, csums=csums, b=b):
                    sc = emit_vdot(b, plc, pens)
                    sl = scores[:, plc * LC:(plc + 1) * LC]
                    nc.scalar.activation(sl, sc, Act.Exp,
                                         bias=negu_sb[:, b:b + 1])
                    # chunk sum on the (otherwise idle) vector engine
                    nc.vector.reduce_sum(csums[:, plc:plc + 1], sl,
                                         axis=mybir.AxisListType.X)

                pending = None  # (lc, ens) awaiting v-dot emission
                for lc in range(NLC):
                    ens = []
                    for ch in range(CH):
                        ps = kpp.tile([P, LC], f32, tag="kp")
                        for ci in range(CI):
                            nc.tensor.matmul(
                                ps,
                                wk_sb[:, ci, ch * P:(ch + 1) * P],
                                kt_slice(kts, ci, lc * LC),
                                start=(ci == 0), stop=(ci == CI - 1))
                        en = enp.tile([P, LC], f32r, tag="en")
                        nc.scalar.activation(en, ps, Act.Tanh,
                                             bias=qpT_sb[:, ch, b:b + 1])
                        ens.append(en)
                    # software-pipeline: emit previous chunk's v-dot after this
                    # chunk's main matmuls so PE never waits on ACT.
                    if pending is not None:
                        finish_chunk(*pending)
                    pending = (lc, ens)
                # prefetch next b's key tiles before the softmax tail
                if b + 1 < BS:
                    next_kts = load_kt(b + 1, [QRT] * 4)
                finish_chunk(*pending)

                # ---- normalize: S = sum of chunk sums, scores *= 1/S ----
                # (halved so the first output DMA overlaps the second mul)
                sums = singles.tile([1, 1], f32, tag=f"sums_{b}")
                inv = singles.tile([1, 1], f32, tag=f"inv_{b}")
                nc.vector.reduce_sum(sums, csums[:, :],
                                     axis=mybir.AxisListType.X)
                nc.vector.reciprocal(inv, sums)
                HL = L // 2
                for h2 in range(2):
                    nc.vector.tensor_scalar_mul(
                        scores[:, h2 * HL:(h2 + 1) * HL],
                        in0=scores[:, h2 * HL:(h2 + 1) * HL], scalar1=inv)
                    nc.sync.dma_start(out[b:b + 1, h2 * HL:(h2 + 1) * HL],
                                      scores[:, h2 * HL:(h2 + 1) * HL])
                if b + 1 < BS:
                    kts = next_kts

    nc.compile()
    return nc


def _get_nc():
    if "nc" not in _CACHE:
        _CACHE["nc"] = _build_nc()
    return _CACHE["nc"]


def _prep_inputs(query, key_val, W, v):
    """Host-side shard prep: returns list of 8 per-core input dicts."""
    query = np.asarray(query, dtype=np.float32)
    key_val = np.asarray(key_val, dtype=np.float32)
    W = np.asarray(W, dtype=np.float32)
    v = np.asarray(v, dtype=np.float32)

    q_proj = (query.astype(np.float64) @ W[:H].astype(np.float64)).astype(np.float32)
    wk_tiled = np.ascontiguousarray(
        W[H:].reshape(CI, P, H).transpose(1, 0, 2))          # [P, CI, H]
    vT_tiled = np.ascontiguousarray(v.reshape(CH, P).T)      # [P, CH]

    # Sample a handful of exact scores per row to place the softmax shift U_b
    # near the row max (any U within ~80 of the max is numerically exact).
    ls = np.linspace(0, L - 1, 64).astype(np.int64)
    kp_s = np.einsum("lbi,ih->lbh", key_val[ls].astype(np.float64),
                     W[H:].astype(np.float64))               # (64, B, H)
    sc_s = np.einsum("h,lbh->bl", v.astype(np.float64),
                     np.tanh(q_proj.astype(np.float64)[None] + kp_s))
    U = sc_s.max(axis=1) + 40.0                              # (B,)

    in_maps = []
    for c in range(NCORES):
        b0 = c * BS
        # key_val[l, b, i] -> [b, ci, p(i), l]
        kt = np.ascontiguousarray(
            key_val[:, b0:b0 + BS, :].transpose(1, 2, 0)
            .reshape(BS, CI, P, L))
        qpT_tiled = np.ascontiguousarray(
            q_proj[b0:b0 + BS].T.reshape(CH, P, BS).transpose(1, 0, 2))
        in_maps.append({
            "keyT": kt,
            "wk": wk_tiled,
            "qpT": qpT_tiled,
            "vT": vT_tiled,
            "negu": np.ascontiguousarray(
                -U[b0:b0 + BS].astype(np.float32).reshape(1, BS)),
        })
    return in_maps


def _run(inputs, trace=False, **trace_kwargs):
    from concourse.bass_utils import run_bass_kernel_spmd

    nc = _get_nc()
    in_maps = _prep_inputs(**inputs)
    res = run_bass_kernel_spmd(
        nc, in_maps, core_ids=list(range(NCORES)), trace=trace, **trace_kwargs)
    out = np.concatenate(
        [np.asarray(r["out"], dtype=np.float32) for r in res.results],
        axis=0).reshape(B, 1, L)
    return out, res


def kernel(**inputs):
    out, _ = _run(inputs, trace=False)
    return out



# revision 6
# speedup vs baseline: 1.1448x; 1.1448x over previous
"""ConcatAttention Trainium2 kernel (8-core data-parallel over batch).

Computes, per batch row b:
    scores[b, l] = sum_h v[h] * tanh(q_proj[b, h] + (key_val[l, b] @ Wk)[h])
    out[b, 0, :] = softmax(scores[b, :])

Device-side per core (B_shard = 4 batch rows):
  - main matmul  kpT[h, l] = Wk^T @ keyT   (float32r, K=512 via 4 PSUM-accum chunks)
  - ACT fuses    energy = tanh(kpT + q_projT[h])  (per-partition bias)
  - v-dot        scores[1, l] = v^T @ energy      (M=1 matmuls, PSUM accum over h,
                 4 l-chunks concurrently in distinct PE column strips via
                 tile_position=(0, 32j) -> ~4x fewer PE-serial cycles)
  - softmax on ACT/DVE, DMA out.

Host side only reshapes/shards: key_val is laid out [b][h_in][L] per core so the
device streams fully contiguous tiles (no on-chip transposes), and the tiny
q_proj = query @ Wq is precomputed on host (it is per-core constant bias data).
"""

import os
import sys

for _p in ("/opt/trn_rl_repo", os.path.expanduser("~/trn_rl_repo")):
    if os.path.isdir(_p) and _p not in sys.path:
        sys.path.insert(0, _p)

import numpy as np

L, B, H = 4096, 32, 512
NCORES = 8
BS = B // NCORES          # batch rows per core
P = 128
CI = H // P               # input-feature chunks (contraction)
CH = H // P               # output-feature chunks
LC = 512                  # l-tile (matmul moving free dim)
NLC = L // LC
QRT = L // 4              # key DMA granularity: [128, QRT] = 512 KiB
WARMUP_MM = 3             # dummy matmul groups to heat the PE HAM clock gate
WAVE = 4                  # v-dot chunks processed concurrently (PE col strips)
WAVE_LAG = 2              # main groups emitted between a wave's last member
                          # and the wave itself (covers ACT tanh latency)

_CACHE = {}


def _build_nc():
    import concourse.bacc as bacc
    import concourse.mybir as mybir
    import concourse.tile as tile

    f32 = mybir.dt.float32
    f32r = mybir.dt.float32r
    f16 = mybir.dt.float16
    Act = mybir.ActivationFunctionType

    nc = bacc.Bacc("TRN2", target_bir_lowering=False)

    keyT = nc.dram_tensor("keyT", [BS, CI, P, L], f32r, kind="ExternalInput")
    wk = nc.dram_tensor("wk", [P, CI, H], f32r, kind="ExternalInput")
    qpT = nc.dram_tensor("qpT", [P, CH, BS], f32, kind="ExternalInput")
    vT = nc.dram_tensor("vT", [P, CH], f16, kind="ExternalInput")
    # -U_b: softmax shift per batch row (host-derived safe bound near the
    # row max; softmax is invariant to the exact value), replicated down
    # all 128 partitions so any PE column strip can use it as ACT bias.
    negu = nc.dram_tensor("negu", [P, BS], f32, kind="ExternalInput")
    out = nc.dram_tensor("out", [BS, L], f32, kind="ExternalOutput")

    with tile.TileContext(nc) as tc:
        with tc.tile_pool(name="singles", bufs=1) as singles, \
             tc.tile_pool(name="ktp", bufs=7) as ktp, \
             tc.tile_pool(name="enp", bufs=24) as enp, \
             tc.tile_pool(name="scrp", bufs=2) as scrp, \
             tc.tile_pool(name="kpp", bufs=6, space="PSUM") as kpp, \
             tc.tile_pool(name="scp", bufs=2, space="PSUM") as scp:

            def load_kt(b, plan, tiles=None, pos=0):
                """plan entries: (width, split_ci). A joint DMA carries all CI
                feature chunks; split_ci issues one DMA per chunk so the first
                matmul can start as soon as its ci lands."""
                if tiles is None:
                    tiles = []
                for w, split in plan:
                    t = ktp.tile([P, CI, QRT], f32r, tag="kt")
                    if split:
                        for ci in range(CI):
                            nc.sync.dma_start(
                                t[:, ci, :w], keyT[b, ci, :, pos:pos + w])
                    else:
                        nc.sync.dma_start(
                            t[:, :, :w],
                            keyT[b, :, :, pos:pos + w].rearrange(
                                "c p l -> p c l"))
                    tiles.append((pos, w, t))
                    pos += w
                return tiles

            def kt_slice(tiles, ci, l0):
                for pos, w, t in tiles:
                    if pos <= l0 and l0 + LC <= pos + w:
                        return t[:, ci, l0 - pos:l0 - pos + LC]
                raise AssertionError("no tile covers slice")

            # ---- constants on the gpsimd queue (per-ci so the first matmul
            # group can start as soon as its chunk lands) ----
            wk_sb = singles.tile([P, CI, H], f32r, tag="wk")
            for ci in range(CI):
                nc.gpsimd.dma_start(wk_sb[:, ci, :], wk[:, ci, :])
            qpT_sb = singles.tile([P, CH, BS], f32, tag="qpT")
            nc.gpsimd.dma_start(qpT_sb, qpT[:, :, :])
            vT_sb = singles.tile([P, CH], f16, tag="vT")
            nc.gpsimd.dma_start(vT_sb, vT[:, :])
            negu_sb = singles.tile([P, BS], f32, tag="negu")
            nc.gpsimd.dma_start(negu_sb, negu[:, :])
            kts = load_kt(0, [(LC, True), (LC, True), (LC, False), (LC, False),
                              (QRT, False), (QRT, False)])

            # ---- PE warmup: cheap dummy matmuls on zeros while the first
            # key tiles stream in, so the HAM clock gate ramps before real
            # work starts ----
            wu = singles.tile([P, LC], f32, tag="warmup")
            nc.vector.memset(wu, 0.0)
            wur = wu[:, :].bitcast(f32r)
            trash = singles.tile([1, 1], f32, tag="trash")
            for g in range(WARMUP_MM):
                wps = kpp.tile([P, LC], f32, tag="kp")
                for i in range(4):
                    nc.tensor.matmul(wps[:, 0:P], wur[:, 0:P], wur[:, 0:P],
                                     start=(i == 0), stop=(i == 3))
                nc.vector.tensor_copy(trash, wps[0:1, 0:1])

            # per-b softmax state
            scores = {}
            csums = {}
            for b in range(BS):
                sc_b = scrp.tile([1, L], f32, tag="scores")
                cs_b = singles.tile([1, NLC], f32, tag=f"csums_{b}")
                scores[b] = sc_b
                csums[b] = cs_b

            def emit_wave(wgroups):
                """wgroups: up to WAVE entries of (b, lc, ens). Emits the
                v-dot matmuls for all entries with each entry in its own PE
                column strip (concurrent execution), then the exp + chunk-sum
                for each entry."""
                scps = scp.tile([P, LC], mybir.dt.float32, tag="sc")
                for ch in range(CH):
                    for j, (b, lc, ens) in enumerate(wgroups):
                        nc.tensor.matmul(
                            scps[32 * j:32 * j + 1, :],
                            vT_sb[:, ch:ch + 1], ens[ch],
                            start=(ch == 0), stop=(ch == CH - 1),
                            tile_position=(0, 32 * j))
                for j, (b, lc, ens) in enumerate(wgroups):
                    sl = scores[b][:, lc * LC:(lc + 1) * LC]
                    nc.scalar.activation(sl, scps[32 * j:32 * j + 1, :],
                                         Act.Exp,
                                         bias=negu_sb[32 * j:32 * j + 1,
                                                      b:b + 1])
                    # chunk sum on the (otherwise idle) vector engine
                    nc.vector.reduce_sum(csums[b][:, lc:lc + 1], sl,
                                         axis=mybir.AxisListType.X)

            def finish_b(b):
                # ---- normalize: S = sum of chunk sums, scores *= 1/S ----
                # (halved so the first output DMA overlaps the second mul)
                sums = singles.tile([1, 1], f32, tag=f"sums_{b}")
                inv = singles.tile([1, 1], f32, tag=f"inv_{b}")
                nc.vector.reduce_sum(sums, csums[b][:, :],
                                     axis=mybir.AxisListType.X)
                nc.vector.reciprocal(inv, sums)
                HL = L // 2
                for h2 in range(2):
                    nc.vector.tensor_scalar_mul(
                        scores[b][:, h2 * HL:(h2 + 1) * HL],
                        in0=scores[b][:, h2 * HL:(h2 + 1) * HL], scalar1=inv)
                    nc.sync.dma_start(out[b:b + 1, h2 * HL:(h2 + 1) * HL],
                                      scores[b][:, h2 * HL:(h2 + 1) * HL])

            # ---- main pipeline ----
            # wave_buf: chunks whose mains+tanh are emitted, awaiting v-dot.
            # Each entry: (global_group_idx, b, lc, ens). A wave fires once
            # WAVE entries exist and the newest is WAVE_LAG groups old.
            wave_buf = []
            done_waves = {b: 0 for b in range(BS)}
            g_idx = 0

            def maybe_emit(g_now, force=False):
                while wave_buf and (
                        (len(wave_buf) >= WAVE
                         and wave_buf[WAVE - 1][0] + WAVE_LAG <= g_now)
                        or (force and wave_buf)):
                    take = wave_buf[:WAVE]
                    del wave_buf[:WAVE]
                    emit_wave([(b, lc, ens) for _, b, lc, ens in take])
                    for _, b, lc, _e in take:
                        done_waves[b] += 1
                        if done_waves[b] == NLC:
                            finish_b(b)

            for b in range(BS):
                for lc in range(NLC):
                    ens = []
                    for ch in range(CH):
                        ps = kpp.tile([P, LC], f32, tag="kp")
                        for ci in range(CI):
                            nc.tensor.matmul(
                                ps,
                                wk_sb[:, ci, ch * P:(ch + 1) * P],
                                kt_slice(kts, ci, lc * LC),
                                start=(ci == 0), stop=(ci == CI - 1))
                        en = enp.tile([P, LC], f16, tag="en")
                        nc.scalar.activation(en, ps, Act.Tanh,
                                             bias=qpT_sb[:, ch, b:b + 1])
                        ens.append(en)
                    wave_buf.append((g_idx, b, lc, ens))
                    g_idx += 1
                    maybe_emit(g_idx)
                # prefetch next b's key tiles before this b's tail work
                if b + 1 < BS:
                    kts = load_kt(b + 1, [(QRT, False)] * 4)
            maybe_emit(g_idx, force=True)

    nc.compile()
    return nc


def _get_nc():
    if "nc" not in _CACHE:
        _CACHE["nc"] = _build_nc()
    return _CACHE["nc"]


def _prep_inputs(query, key_val, W, v):
    """Host-side shard prep: returns list of 8 per-core input dicts."""
    query = np.asarray(query, dtype=np.float32)
    key_val = np.asarray(key_val, dtype=np.float32)
    W = np.asarray(W, dtype=np.float32)
    v = np.asarray(v, dtype=np.float32)

    q_proj = (query.astype(np.float64) @ W[:H].astype(np.float64)).astype(np.float32)
    wk_tiled = np.ascontiguousarray(
        W[H:].reshape(CI, P, H).transpose(1, 0, 2))          # [P, CI, H]
    vT_tiled = np.ascontiguousarray(v.reshape(CH, P).T)      # [P, CH]

    # Sample a handful of exact scores per row to place the softmax shift U_b
    # near the row max (any U within ~80 of the max is numerically exact).
    ls = np.linspace(0, L - 1, 64).astype(np.int64)
    kp_s = np.einsum("lbi,ih->lbh", key_val[ls].astype(np.float64),
                     W[H:].astype(np.float64))               # (64, B, H)
    sc_s = np.einsum("h,lbh->bl", v.astype(np.float64),
                     np.tanh(q_proj.astype(np.float64)[None] + kp_s))
    U = sc_s.max(axis=1) + 40.0                              # (B,)

    in_maps = []
    for c in range(NCORES):
        b0 = c * BS
        # key_val[l, b, i] -> [b, ci, p(i), l]
        kt = np.ascontiguousarray(
            key_val[:, b0:b0 + BS, :].transpose(1, 2, 0)
            .reshape(BS, CI, P, L))
        qpT_tiled = np.ascontiguousarray(
            q_proj[b0:b0 + BS].T.reshape(CH, P, BS).transpose(1, 0, 2))
        negu_rep = np.ascontiguousarray(
            np.tile(-U[b0:b0 + BS].astype(np.float32).reshape(1, BS),
                    (P, 1)))
        in_maps.append({
            "keyT": kt,
            "wk": wk_tiled,
            "qpT": qpT_tiled,
            "vT": vT_tiled.astype(np.float16),
            "negu": negu_rep,
        })
    return in_maps


def _run(inputs, trace=False, **trace_kwargs):
    from concourse.bass_utils import run_bass_kernel_spmd

    nc = _get_nc()
    in_maps = _prep_inputs(**inputs)
    res = run_bass_kernel_spmd(
        nc, in_maps, core_ids=list(range(NCORES)), trace=trace, **trace_kwargs)
    out = np.concatenate(
        [np.asarray(r["out"], dtype=np.float32) for r in res.results],
        axis=0).reshape(B, 1, L)
    return out, res


def kernel(**inputs):
    out, _ = _run(inputs, trace=False)
    return out
